# revision 22
# baseline (speedup 1.0000x reference)
"""Multi-head attention (b=4, n=2048, d=1024, 16 heads) on 8 TRN2 NeuronCores.

Sharding: core c handles batch b=c//2, head-group g=c%2 (8 heads each).
Each core computes its head-group's attention output projected through its
row-slice of Wo; the host sums the two partial projections per batch and
adds the bias (the tensor-parallel all-reduce, done at gather time).

The kernel is organized as one fused software pipeline. The ScalarE exp of
the attention scores is the hard floor (256 activations x ~1.1us); every
other engine's work is scheduled into the PE/DVE slack underneath it:

  per (i-tile, head-pair) block, j in 0..15:
      scores.T[j,i] per head via row-tiled K=64 matmul pairs -> PSUM
      exp via ScalarE (scale folded) -> SBUF bf16
      out.T accumulated in PSUM (augmented-V ones column yields softmax
      denominators for free)
      + "filler" units (projection matmul chain steps, PSUM->SBUF casts,
        norm/proj of earlier blocks) popped from a deadline-ordered queue

All matmuls run in bf16 (f32r streams ~1.5x slower on the PE; the error
budget allows bf16 everywhere), so x is shipped once as bf16 and the fp32
copy is never needed.
"""

import numpy as np
import ml_dtypes

import concourse.bass as bass
import concourse.tile as tile
from concourse import bacc, mybir
import concourse.bass_utils as bass_utils

F32 = mybir.dt.float32
F32R = mybir.dt.float32r
BF16 = mybir.dt.bfloat16
FP8 = mybir.dt.float8e4
DR = mybir.MatmulPerfMode.DoubleRow
EXP = mybir.ActivationFunctionType.Exp

# Q/K projections run in fp8e4m3 with DoubleRow (2 fp8 MACs/cell/cycle).
# e4m3's normal range bottoms out at 2^-6, so Wq/Wk (sigma=0.02) are
# pre-scaled by 64 on the host; the 1/(64*64) comes out in the softmax
# scale folded into the exp activation.
W8SCALE = 64.0

B, N, D = 4, 2048, 1024
HEADS, HD = 16, 64
GROUPS = 2            # head groups (tensor-parallel dimension)
GH = HEADS // GROUPS  # 8 heads per group
PAIRS = GH // 2       # 4 head pairs per core
DG = GH * HD          # 512 columns per group
KT = D // 128         # 8 contraction tiles
NT = N // 128         # 16 key tiles
IT = N // 512         # 4 query i-tiles
SCALE = float(D) ** -0.5

_CACHE = {}


def _build_kernel():
    nc = bacc.Bacc("TRN2", target_bir_lowering=False, debug=False, num_devices=8)

    xbf_d = nc.dram_tensor("xbf", [D, N], BF16, kind="ExternalInput").ap()
    xf8_d = nc.dram_tensor("xf8", [D, N], FP8, kind="ExternalInput").ap()
    wq_d = nc.dram_tensor("wq", [D, DG], FP8, kind="ExternalInput").ap()
    wk_d = nc.dram_tensor("wk", [D, DG], FP8, kind="ExternalInput").ap()
    wv_d = nc.dram_tensor("wv", [D, DG], BF16, kind="ExternalInput").ap()
    wo_d = nc.dram_tensor("wo", [DG, D], BF16, kind="ExternalInput").ap()
    vtpl_d = nc.dram_tensor("vtpl", [128, NT, PAIRS, 64], BF16, kind="ExternalInput").ap()
    sel_d = nc.dram_tensor("sel", [65, 128], F32R, kind="ExternalInput").ap()
    zer_d = nc.dram_tensor("zer", [65, 512], F32R, kind="ExternalInput").ap()
    y_d = nc.dram_tensor("y", [N, D], F32, kind="ExternalOutput").ap()

    with tile.TileContext(nc) as tc:
        with (
            tc.tile_pool(name="sb", bufs=1) as sb,
            tc.tile_pool(name="sb2", bufs=2) as sb2,
            tc.tile_pool(name="sb3", bufs=3) as sb3,
            tc.tile_pool(name="ps_sc", bufs=2, space="PSUM") as ps_sc,
            tc.tile_pool(name="ps_ot", bufs=1, space="PSUM") as ps_ot,
            tc.tile_pool(name="ps_ms", bufs=2, space="PSUM") as ps_ms,
        ):
            # ---- persistent SBUF ----
            xbf = sb.tile([128, KT, N], BF16)
            xf8 = sb.tile([128, KT, N], FP8)
            wq = sb.tile([128, KT, DG], FP8)
            wk = sb.tile([128, KT, DG], FP8)
            wv = sb.tile([128, KT, DG], BF16)
            wo = sb.tile([128, PAIRS, D], BF16)
            kT = sb.tile([128, PAIRS, N], BF16)
            # per (j-tile, pair): [V_even(64) | ones(1) | zeros(63) | V_odd(64)]
            # A-lhsT = cols 0:128, B-lhsT = cols 64:192 (ones+zeros shared)
            vaug = sb.tile([128, NT, PAIRS, 192], BF16)
            srow = sb.tile([65, 512], F32R)   # rows 0/64 carry softmax sums
            selt = sb.tile([65, 128], F32R)

            # Inputs split across the two HWDGE queues (sync + scalar) so
            # the head loads in parallel: sync carries the lead-in's fp8
            # q/k path, scalar carries the bf16 V path + consts.
            nc.sync.dma_start(
                xf8[:, :, bass.ts(0, 512)],
                xf8_d[:, bass.ts(0, 512)].rearrange("(kt p) n -> p kt n", p=128),
            )
            nc.sync.dma_start(wk[:], wk_d.rearrange("(kt p) m -> p kt m", p=128))
            nc.sync.dma_start(wq[:], wq_d.rearrange("(kt p) m -> p kt m", p=128))
            nc.scalar.dma_start(srow[:], zer_d)
            nc.scalar.dma_start(selt[:], sel_d)
            nc.scalar.dma_start(vaug[:, :, :, 64:128], vtpl_d)
            nc.scalar.dma_start(wv[:], wv_d.rearrange("(kt p) m -> p kt m", p=128))
            nc.scalar.dma_start(
                xbf[:, :, bass.ts(0, 512)],
                xbf_d[:, bass.ts(0, 512)].rearrange("(kt p) n -> p kt n", p=128),
            )
            # bulk loads consumed later ride the idle gpsimd SWDGE queue
            for it in range(1, IT):
                nc.gpsimd.dma_start(
                    xbf[:, :, bass.ts(it, 512)],
                    xbf_d[:, bass.ts(it, 512)].rearrange("(kt p) n -> p kt n", p=128),
                )
                nc.gpsimd.dma_start(
                    xf8[:, :, bass.ts(it, 512)],
                    xf8_d[:, bass.ts(it, 512)].rearrange("(kt p) n -> p kt n", p=128),
                )
            nc.gpsimd.dma_start(wo[:], wo_d.rearrange("(pr p) m -> p pr m", p=128))

            # ---------- emission helpers ----------
            def emit_kt_chain(p, itc):
                # kT[:, p, itc*512:+512] = (wk-slice).T @ x-cols; fp8
                # DoubleRow contracts two 128-deep k-subtiles per matmul
                cell = {}
                def mm(k):
                    if "t" not in cell:
                        cell["t"] = ps_ms.tile([128, 512], F32, tag="misc", name="mischain")
                    nc.tensor.matmul(
                        cell["t"][:], wk[:, 2 * k:2 * k + 2, bass.ts(p, 128)],
                        xf8[:, 2 * k:2 * k + 2, bass.ts(itc, 512)],
                        start=(k == 0), stop=(k == KT // 2 - 1),
                        perf_mode=DR,
                    )
                def cast():
                    nc.vector.tensor_copy(kT[:, p, bass.ts(itc, 512)], cell["t"][:])
                return [(260, (lambda k=k: mm(k))) for k in range(KT // 2)] + [(0, cast)]

            def emit_q_chain(it, p, qcell):
                def mm(k):
                    if "t" not in qcell:
                        qcell["t"] = ps_ms.tile([128, 512], F32, tag="misc", name="mischain")
                    nc.tensor.matmul(
                        qcell["t"][:], wq[:, 2 * k:2 * k + 2, bass.ts(p, 128)],
                        xf8[:, 2 * k:2 * k + 2, bass.ts(it, 512)],
                        start=(k == 0), stop=(k == KT // 2 - 1),
                        perf_mode=DR,
                    )
                def cast():
                    qb = sb2.tile([128, 512], BF16, tag="qb", name="qb")
                    nc.vector.tensor_copy(qb[:], qcell["t"][:])
                    qcell["qb"] = qb
                return [(260, (lambda k=k: mm(k))) for k in range(KT // 2)] + [(0, cast)]

            def emit_v_chain(nt, half):
                # V for tokens nt*128:+128, dims of pairs (2h, 2h+1)  (10 units)
                cell = {}
                def mm(k):
                    if "t" not in cell:
                        cell["t"] = ps_ms.tile([128, 256], F32, tag="misc", name="vchain")
                    nc.tensor.matmul(
                        cell["t"][:], xbf[:, k, bass.ts(nt, 128)],
                        wv[:, k, bass.ds(half * 256, 256)],
                        start=(k == 0), stop=(k == KT - 1),
                    )
                def cast():
                    vr = cell["t"].rearrange("p (pr c) -> p pr c", pr=2)
                    with nc.allow_low_precision(reason="bf16 V tiles"):
                        nc.vector.tensor_copy(
                            vaug[:, nt, 2 * half:2 * half + 2, 0:64], vr[:, :, 0:64]
                        )
                        nc.vector.tensor_copy(
                            vaug[:, nt, 2 * half:2 * half + 2, 128:192],
                            vr[:, :, 64:128],
                        )
                return [(120, (lambda k=k: mm(k))) for k in range(KT)] + [(0, cast)]

            def emit_norm_units(otA, otB, otn, p):
                cell = {}
                def sel_mm():
                    cell["b"] = ps_ms.tile([128, 512], F32, tag="misc", name="bps")
                    nc.tensor.matmul(cell["b"][:], selt[:], srow[:],
                                     start=True, stop=True)
                def recip():
                    cell["r"] = sb2.tile([128, 512], F32, tag="rb", name="rb")
                    nc.vector.reciprocal_approx_fast(cell["r"][:], cell["b"][:])
                def mulA():
                    with nc.allow_low_precision(reason="bf16 normalized attn out"):
                        nc.vector.tensor_mul(
                            out=otn[0:64, p, :], in0=otA[0:64, :],
                            in1=cell["r"][0:64, :],
                        )
                def mulB():
                    with nc.allow_low_precision(reason="bf16 normalized attn out"):
                        nc.vector.tensor_mul(
                            out=otn[64:128, p, :], in0=otB[64:128, :],
                            in1=cell["r"][64:128, :],
                        )
                return [(230, sel_mm), (0, recip), (0, mulA), (0, mulB)]

            def emit_yproj_chain(it, otn, isub, do):
                cell = {}
                def mm(p):
                    if "t" not in cell:
                        cell["t"] = ps_ms.tile([128, 512], F32, tag="misc", name="mischain")
                    nc.tensor.matmul(
                        cell["t"][:], otn[:, p, bass.ts(isub, 128)],
                        wo[:, p, bass.ts(do, 512)],
                        start=(p == 0), stop=(p == PAIRS - 1),
                    )
                def fin():
                    yo = sb2.tile([128, 512], F32, tag="yo", name="yo")
                    nc.vector.tensor_copy(yo[:], cell["t"][:])
                    nc.sync.dma_start(
                        y_d[bass.ds(it * 512 + isub * 128, 128),
                            bass.ts(do, 512)],
                        yo[:],
                    )
                return [(230, (lambda p=p: mm(p))) for p in range(PAIRS)] + [(0, fin)]

            # ---------- deadline queue ----------
            # entry = [deadline_g, units, pos]; g = global iter = block*16 + j.
            # Chains (unit lists sharing one PSUM-pool tile) are popped
            # strictly one at a time: a chain may pause mid-way between
            # sched points, but no new chain starts until the open one
            # closes. Interleaving two pool chains inverts the engine-queue
            # order of their tile-slot release (cast) vs acquire (matmul)
            # and deadlocks the whole pipeline.
            queue = []
            sched = {"open": None}

            def add(g, units):
                queue.append([g, list(units), 0])
                queue.sort(key=lambda e: e[0])

            def _drain(e):
                while e[2] < len(e[1]):
                    _, fn = e[1][e[2]]
                    e[2] += 1
                    fn()

            # q cells per block, created up-front so scores can find qb
            qcells = [[{} for _ in range(PAIRS)] for _ in range(IT)]

            # vaug chunks nt=1..15 during block 0 (nt0 is lead-in)
            for nt in range(1, NT):
                for half in range(2):
                    add(nt - 1, emit_v_chain(nt, half))
            # kT chains: (p0,it0) is lead-in
            for itc in range(1, IT):
                add(itc * 4 - 2, emit_kt_chain(0, itc))
            for p in range(1, PAIRS):
                add(p * 16 - 6, emit_kt_chain(p, 0))
                for itc in range(1, IT):
                    add(p * 16 + itc * 4 - 2, emit_kt_chain(p, itc))
            # q chains for blocks b>0
            for it in range(IT):
                for p in range(PAIRS):
                    b = it * 4 + p
                    if b == 0:
                        continue
                    add(b * 16 - 8, emit_q_chain(it, p, qcells[it][p]))

            def sched_point(g, budget):
                # forced: fully drain every due chain (flushing the open one
                # first so chains never interleave). The open chain's own
                # deadline counts too -- a budget-opened chain otherwise
                # trickles out at budget rate and misses its consumers.
                if sched["open"] is not None and sched["open"][0] <= g:
                    _drain(sched["open"])
                    sched["open"] = None
                while queue and queue[0][0] <= g:
                    if sched["open"] is not None:
                        _drain(sched["open"])
                        sched["open"] = None
                    _drain(queue.pop(0))
                # budgeted: continue/open chains, may pause mid-chain.
                # Never OPEN a chain more than one block ahead of its
                # deadline: pool buffers (qb/yo/otn, 2-deep) must not be
                # re-granted before the previous generation's readers have
                # even been emitted.
                while budget > 0:
                    e = sched["open"]
                    if e is None:
                        if not queue or queue[0][0] > g + 16:
                            break
                        e = queue.pop(0)
                        sched["open"] = e
                    while budget > 0 and e[2] < len(e[1]):
                        cost, fn = e[1][e[2]]
                        e[2] += 1
                        fn()
                        budget -= cost
                    if e[2] == len(e[1]):
                        sched["open"] = None
                    else:
                        break

            # ---------- lead-in ----------
            for cost, fn in emit_kt_chain(0, 0):
                fn()
            for cost, fn in emit_q_chain(0, 0, qcells[0][0]):
                fn()
            for half in range(2):
                for cost, fn in emit_v_chain(0, half):
                    fn()

            # ---------- main fused loop ----------
            for it in range(IT):
                otn = sb2.tile([128, PAIRS, 512], BF16, tag="otn")
                for p in range(PAIRS):
                    b = it * 4 + p
                    qb = qcells[it][p]["qb"]
                    otA = ps_ot.tile([128, 512], F32, tag="otA")
                    otB = ps_ot.tile([128, 512], F32, tag="otB")

                    def scores(j, p=p, qb=qb):
                        stp = ps_sc.tile([128, 1024], F32, tag="sc", name="stp")
                        nc.tensor.matmul(
                            stp[:, 0:512], kT[0:64, p, bass.ts(j, 128)],
                            qb[0:64, :], start=True, stop=True,
                            tile_position=(0, 0),
                        )
                        nc.tensor.matmul(
                            stp[:, 512:1024], kT[64:128, p, bass.ts(j, 128)],
                            qb[64:128, :], start=True, stop=True,
                            tile_position=(64, 0),
                        )
                        ex = sb3.tile([128, 1024], BF16, tag="ex", name="ex")
                        with nc.allow_low_precision(reason="bf16 attn weights"):
                            nc.scalar.activation(ex[:], stp[:], EXP, scale=SCALE / (W8SCALE * W8SCALE))
                        return ex

                    ex_cur = scores(0)
                    # force due units (previous block's norm chain, late
                    # projections) BEFORE this block's first attnV: attnV j0
                    # overwrites otA/otB, which the norm must read first
                    sched_point(b * 16 - 1, 0)
                    for j in range(NT):
                        ex_nxt = scores(j + 1) if j + 1 < NT else None
                        nc.tensor.matmul(
                            otA[:, :], vaug[:, j, p, 0:128], ex_cur[:, 0:512],
                            start=(j == 0), stop=(j == NT - 1),
                        )
                        nc.tensor.matmul(
                            otB[:, :], vaug[:, j, p, 64:192], ex_cur[:, 512:1024],
                            start=(j == 0), stop=(j == NT - 1),
                        )
                        ex_cur = ex_nxt
                        sched_point(b * 16 + j, 560)

                    # softmax denominators -> srow (rows 64 / 0)
                    with nc.allow_low_precision(reason="f32r softmax sums"):
                        nc.vector.tensor_copy(srow[64:65, :], otA[64:65, :])
                        nc.vector.tensor_copy(srow[0:1, :], otB[0:1, :])
                    # norm reads otA/otB (single-buffered) -> force at the
                    # start of the next block, before its attnV j0 lands
                    add(b * 16 + 15, emit_norm_units(otA, otB, otn, p))

                # spread the output projection's 8 sub-chains across the
                # following blocks' slack instead of bursting at one
                # deadline. +16 keeps the earliest budget pop (deadline -
                # lookahead) after the forced norm(it, p3) that writes
                # otn[:, 3, :] at the next block's pre-attnV point.
                for idx, (isub, do) in enumerate(
                    (i, d) for i in range(4) for d in range(2)
                ):
                    add((it + 1) * 64 + 16 + 5 * idx,
                        emit_yproj_chain(it, otn, isub, do))

            # ---------- flush ----------
            if sched["open"] is not None:
                _drain(sched["open"])
                sched["open"] = None
            while queue:
                _drain(queue.pop(0))

    nc.compile()
    return nc


def _host_consts():
    # vaug cols 64:128 = [ones | zeros*63], shared by both heads' lhsT views
    vtpl = np.zeros((128, NT, PAIRS, 64), dtype=ml_dtypes.bfloat16)
    vtpl[:, :, :, 0] = 1.0
    sel = np.zeros((65, 128), dtype=np.float32)
    sel[64, 0:64] = 1.0     # rows 0-63  <- sums(even head)  (srow row 64)
    sel[0, 64:128] = 1.0    # rows 64-127 <- sums(odd head)  (srow row 0)
    zer = np.zeros((65, 512), dtype=np.float32)
    return vtpl, sel, zer


def kernel(x, Wq, Wk, Wv, Wo, bo, _run_kwargs=None):
    x = np.asarray(x, dtype=np.float32)
    Wq = np.asarray(Wq, dtype=np.float32)
    Wk = np.asarray(Wk, dtype=np.float32)
    Wv = np.asarray(Wv, dtype=np.float32)
    Wo = np.asarray(Wo, dtype=np.float32)
    bo = np.asarray(bo, dtype=np.float32)

    if "nc" not in _CACHE:
        _CACHE["nc"] = _build_kernel()
    nc = _CACHE["nc"]

    vtpl, sel, zer = _host_consts()
    in_maps = []
    for c in range(8):
        b, g = c // 2, c % 2
        xt = np.ascontiguousarray(x[b].T)
        cols = slice(g * DG, (g + 1) * DG)
        in_maps.append({
            "xbf": xt.astype(ml_dtypes.bfloat16),
            "xf8": xt.astype(ml_dtypes.float8_e4m3),
            "wq": np.ascontiguousarray(Wq[:, cols] * W8SCALE).astype(ml_dtypes.float8_e4m3),
            "wk": np.ascontiguousarray(Wk[:, cols] * W8SCALE).astype(ml_dtypes.float8_e4m3),
            "wv": np.ascontiguousarray(Wv[:, cols]).astype(ml_dtypes.bfloat16),
            "wo": np.ascontiguousarray(Wo[cols, :]).astype(ml_dtypes.bfloat16),
            "vtpl": vtpl,
            "sel": sel,
            "zer": zer,
        })

    res = bass_utils.run_bass_kernel_spmd(
        nc, in_maps, core_ids=list(range(8)), **(_run_kwargs or {})
    )
    if _run_kwargs:
        _CACHE["last_results"] = res

    y = np.empty((B, N, D), dtype=np.float32)
    for b in range(B):
        y[b] = res.results[2 * b]["y"] + res.results[2 * b + 1]["y"] + bo
    return y


# revision 23
# speedup vs baseline: 1.0420x; 1.0420x over previous
"""Multi-head attention (b=4, n=2048, d=1024, 16 heads) on 8 TRN2 NeuronCores.

Sharding: core c handles batch b=c//2, head-group g=c%2 (8 heads each).
Each core computes its head-group's attention output projected through its
row-slice of Wo; the host sums the two partial projections per batch and
adds the bias (the tensor-parallel all-reduce, done at gather time).

The kernel is organized as one fused software pipeline. The ScalarE exp of
the attention scores is the hard floor (256 activations x ~1.1us); every
other engine's work is scheduled into the PE/DVE slack underneath it:

  per (i-tile, head-pair) block, j in 0..15:
      scores.T[j,i] per head via row-tiled K=64 matmul pairs -> PSUM
      exp via ScalarE (scale folded) -> SBUF bf16
      out.T accumulated in PSUM (augmented-V ones column yields softmax
      denominators for free)
      + "filler" units (projection matmul chain steps, PSUM->SBUF casts,
        norm/proj of earlier blocks) popped from a deadline-ordered queue

All matmuls run in bf16 (f32r streams ~1.5x slower on the PE; the error
budget allows bf16 everywhere), so x is shipped once as bf16 and the fp32
copy is never needed.
"""

import numpy as np
import ml_dtypes

import concourse.bass as bass
import concourse.tile as tile
from concourse import bacc, mybir
import concourse.bass_utils as bass_utils

F32 = mybir.dt.float32
F32R = mybir.dt.float32r
BF16 = mybir.dt.bfloat16
FP8 = mybir.dt.float8e4
DR = mybir.MatmulPerfMode.DoubleRow
EXP = mybir.ActivationFunctionType.Exp

# Q/K projections run in fp8e4m3 with DoubleRow (2 fp8 MACs/cell/cycle).
# e4m3's normal range bottoms out at 2^-6, so Wq/Wk (sigma=0.02) are
# pre-scaled by 64 on the host; the 1/(64*64) comes out in the softmax
# scale folded into the exp activation.
W8SCALE = 64.0

B, N, D = 4, 2048, 1024
HEADS, HD = 16, 64
GROUPS = 2            # head groups (tensor-parallel dimension)
GH = HEADS // GROUPS  # 8 heads per group
PAIRS = GH // 2       # 4 head pairs per core
DG = GH * HD          # 512 columns per group
KT = D // 128         # 8 contraction tiles
NT = N // 128         # 16 key tiles
IT = N // 512         # 4 query i-tiles
SCALE = float(D) ** -0.5

_CACHE = {}


def _build_kernel():
    nc = bacc.Bacc("TRN2", target_bir_lowering=False, debug=False, num_devices=8)

    xbf_d = nc.dram_tensor("xbf", [D, N], BF16, kind="ExternalInput").ap()
    xf8_d = nc.dram_tensor("xf8", [D, N], FP8, kind="ExternalInput").ap()
    wq_d = nc.dram_tensor("wq", [D, DG], FP8, kind="ExternalInput").ap()
    wk_d = nc.dram_tensor("wk", [D, DG], FP8, kind="ExternalInput").ap()
    wv_d = nc.dram_tensor("wv", [D, DG], BF16, kind="ExternalInput").ap()
    wo_d = nc.dram_tensor("wo", [DG, D], BF16, kind="ExternalInput").ap()
    vtpl_d = nc.dram_tensor("vtpl", [128, NT, PAIRS, 64], BF16, kind="ExternalInput").ap()
    sel_d = nc.dram_tensor("sel", [65, 128], F32R, kind="ExternalInput").ap()
    zer_d = nc.dram_tensor("zer", [65, 512], F32R, kind="ExternalInput").ap()
    y_d = nc.dram_tensor("y", [N, D], F32, kind="ExternalOutput").ap()

    with tile.TileContext(nc) as tc:
        with (
            tc.tile_pool(name="sb", bufs=1) as sb,
            tc.tile_pool(name="sb2", bufs=2) as sb2,
            tc.tile_pool(name="sb3", bufs=3) as sb3,
            tc.tile_pool(name="ps_sc", bufs=2, space="PSUM") as ps_sc,
            tc.tile_pool(name="ps_ot", bufs=1, space="PSUM") as ps_ot,
            tc.tile_pool(name="ps_ms", bufs=2, space="PSUM") as ps_ms,
        ):
            # ---- persistent SBUF ----
            xbf = sb.tile([128, KT, N], BF16)
            xf8 = sb.tile([128, KT, N], FP8)
            wq = sb.tile([128, KT, DG], FP8)
            wk = sb.tile([128, KT, DG], FP8)
            wv = sb.tile([128, KT, DG], BF16)
            wo = sb.tile([128, PAIRS, D], BF16)
            kT = sb.tile([128, PAIRS, N], BF16)
            # per (j-tile, pair): [V_even(64) | ones(1) | zeros(63) | V_odd(64)]
            # A-lhsT = cols 0:128, B-lhsT = cols 64:192 (ones+zeros shared)
            vaug = sb.tile([128, NT, PAIRS, 192], BF16)
            srow = sb.tile([65, 512], F32R)   # rows 0/64 carry softmax sums
            selt = sb.tile([65, 128], F32R)

            # Two parallel input streams. NOTHING loads via nc.scalar --
            # a DMA on the Activation queue occupies the bottleneck engine
            # for the whole transfer. sync: the fp8 q/k lead-in path;
            # gpsimd SWDGE: the bf16 V path and bulk chunks.
            nc.sync.dma_start(srow[:], zer_d)
            nc.sync.dma_start(selt[:], sel_d)
            nc.sync.dma_start(
                xf8[:, :, bass.ts(0, 512)],
                xf8_d[:, bass.ts(0, 512)].rearrange("(kt p) n -> p kt n", p=128),
            )
            nc.sync.dma_start(wk[:], wk_d.rearrange("(kt p) m -> p kt m", p=128))
            nc.sync.dma_start(wq[:], wq_d.rearrange("(kt p) m -> p kt m", p=128))
            nc.gpsimd.dma_start(vaug[:, :, :, 64:128], vtpl_d)
            nc.gpsimd.dma_start(wv[:], wv_d.rearrange("(kt p) m -> p kt m", p=128))
            nc.gpsimd.dma_start(
                xbf[:, :, bass.ts(0, 512)],
                xbf_d[:, bass.ts(0, 512)].rearrange("(kt p) n -> p kt n", p=128),
            )
            for it in range(1, IT):
                nc.sync.dma_start(
                    xf8[:, :, bass.ts(it, 512)],
                    xf8_d[:, bass.ts(it, 512)].rearrange("(kt p) n -> p kt n", p=128),
                )
                nc.gpsimd.dma_start(
                    xbf[:, :, bass.ts(it, 512)],
                    xbf_d[:, bass.ts(it, 512)].rearrange("(kt p) n -> p kt n", p=128),
                )
            nc.gpsimd.dma_start(wo[:], wo_d.rearrange("(pr p) m -> p pr m", p=128))

            # ---------- emission helpers ----------
            def emit_kt_chain(p, itc):
                # kT[:, p, itc*512:+512] = (wk-slice).T @ x-cols; fp8
                # DoubleRow contracts two 128-deep k-subtiles per matmul
                cell = {}
                def mm(k):
                    if "t" not in cell:
                        cell["t"] = ps_ms.tile([128, 512], F32, tag="misc", name="mischain")
                    nc.tensor.matmul(
                        cell["t"][:], wk[:, 2 * k:2 * k + 2, bass.ts(p, 128)],
                        xf8[:, 2 * k:2 * k + 2, bass.ts(itc, 512)],
                        start=(k == 0), stop=(k == KT // 2 - 1),
                        perf_mode=DR,
                    )
                def cast():
                    nc.vector.tensor_copy(kT[:, p, bass.ts(itc, 512)], cell["t"][:])
                return [(260, (lambda k=k: mm(k))) for k in range(KT // 2)] + [(0, cast)]

            def emit_q_chain(it, p, qcell):
                def mm(k):
                    if "t" not in qcell:
                        qcell["t"] = ps_ms.tile([128, 512], F32, tag="misc", name="mischain")
                    nc.tensor.matmul(
                        qcell["t"][:], wq[:, 2 * k:2 * k + 2, bass.ts(p, 128)],
                        xf8[:, 2 * k:2 * k + 2, bass.ts(it, 512)],
                        start=(k == 0), stop=(k == KT // 2 - 1),
                        perf_mode=DR,
                    )
                def cast():
                    qb = sb2.tile([128, 512], BF16, tag="qb", name="qb")
                    nc.vector.tensor_copy(qb[:], qcell["t"][:])
                    qcell["qb"] = qb
                return [(260, (lambda k=k: mm(k))) for k in range(KT // 2)] + [(0, cast)]

            def emit_v_chain(nt, half):
                # V for tokens nt*128:+128, dims of pairs (2h, 2h+1)  (10 units)
                cell = {}
                def mm(k):
                    if "t" not in cell:
                        cell["t"] = ps_ms.tile([128, 256], F32, tag="misc", name="vchain")
                    nc.tensor.matmul(
                        cell["t"][:], xbf[:, k, bass.ts(nt, 128)],
                        wv[:, k, bass.ds(half * 256, 256)],
                        start=(k == 0), stop=(k == KT - 1),
                    )
                def cast():
                    vr = cell["t"].rearrange("p (pr c) -> p pr c", pr=2)
                    with nc.allow_low_precision(reason="bf16 V tiles"):
                        nc.vector.tensor_copy(
                            vaug[:, nt, 2 * half:2 * half + 2, 0:64], vr[:, :, 0:64]
                        )
                        nc.vector.tensor_copy(
                            vaug[:, nt, 2 * half:2 * half + 2, 128:192],
                            vr[:, :, 64:128],
                        )
                return [(120, (lambda k=k: mm(k))) for k in range(KT)] + [(0, cast)]

            def emit_norm_units(otA, otB, otn, p):
                cell = {}
                def sel_mm():
                    cell["b"] = ps_ms.tile([128, 512], F32, tag="misc", name="bps")
                    nc.tensor.matmul(cell["b"][:], selt[:], srow[:],
                                     start=True, stop=True)
                def recip():
                    cell["r"] = sb2.tile([128, 512], F32, tag="rb", name="rb")
                    nc.vector.reciprocal_approx_fast(cell["r"][:], cell["b"][:])
                def mulA():
                    with nc.allow_low_precision(reason="bf16 normalized attn out"):
                        nc.vector.tensor_mul(
                            out=otn[0:64, p, :], in0=otA[0:64, :],
                            in1=cell["r"][0:64, :],
                        )
                def mulB():
                    with nc.allow_low_precision(reason="bf16 normalized attn out"):
                        nc.vector.tensor_mul(
                            out=otn[64:128, p, :], in0=otB[64:128, :],
                            in1=cell["r"][64:128, :],
                        )
                return [(230, sel_mm), (0, recip), (0, mulA), (0, mulB)]

            def emit_yproj_chain(it, otn, isub, do):
                cell = {}
                def mm(p):
                    if "t" not in cell:
                        cell["t"] = ps_ms.tile([128, 512], F32, tag="misc", name="mischain")
                    nc.tensor.matmul(
                        cell["t"][:], otn[:, p, bass.ts(isub, 128)],
                        wo[:, p, bass.ts(do, 512)],
                        start=(p == 0), stop=(p == PAIRS - 1),
                    )
                def fin():
                    yo = sb2.tile([128, 512], F32, tag="yo", name="yo")
                    nc.vector.tensor_copy(yo[:], cell["t"][:])
                    nc.sync.dma_start(
                        y_d[bass.ds(it * 512 + isub * 128, 128),
                            bass.ts(do, 512)],
                        yo[:],
                    )
                return [(230, (lambda p=p: mm(p))) for p in range(PAIRS)] + [(0, fin)]

            # ---------- deadline queue ----------
            # entry = [deadline_g, units, pos]; g = global iter = block*16 + j.
            # Chains (unit lists sharing one PSUM-pool tile) are popped
            # strictly one at a time: a chain may pause mid-way between
            # sched points, but no new chain starts until the open one
            # closes. Interleaving two pool chains inverts the engine-queue
            # order of their tile-slot release (cast) vs acquire (matmul)
            # and deadlocks the whole pipeline.
            queue = []
            sched = {"open": None}

            def add(g, units):
                queue.append([g, list(units), 0])
                queue.sort(key=lambda e: e[0])

            def _drain(e):
                while e[2] < len(e[1]):
                    _, fn = e[1][e[2]]
                    e[2] += 1
                    fn()

            # q cells per block, created up-front so scores can find qb
            qcells = [[{} for _ in range(PAIRS)] for _ in range(IT)]

            # vaug chunks nt=1..15 during block 0 (nt0 is lead-in)
            for nt in range(1, NT):
                for half in range(2):
                    add(nt - 1, emit_v_chain(nt, half))
            # kT chains: (p0,it0) is lead-in
            for itc in range(1, IT):
                add(itc * 4 - 2, emit_kt_chain(0, itc))
            for p in range(1, PAIRS):
                add(p * 16 - 6, emit_kt_chain(p, 0))
                for itc in range(1, IT):
                    add(p * 16 + itc * 4 - 2, emit_kt_chain(p, itc))
            # q chains for blocks b>0
            for it in range(IT):
                for p in range(PAIRS):
                    b = it * 4 + p
                    if b == 0:
                        continue
                    add(b * 16 - 8, emit_q_chain(it, p, qcells[it][p]))

            def sched_point(g, budget):
                # forced: fully drain every due chain (flushing the open one
                # first so chains never interleave). The open chain's own
                # deadline counts too -- a budget-opened chain otherwise
                # trickles out at budget rate and misses its consumers.
                if sched["open"] is not None and sched["open"][0] <= g:
                    _drain(sched["open"])
                    sched["open"] = None
                while queue and queue[0][0] <= g:
                    if sched["open"] is not None:
                        _drain(sched["open"])
                        sched["open"] = None
                    _drain(queue.pop(0))
                # budgeted: continue/open chains, may pause mid-chain.
                # Never OPEN a chain more than one block ahead of its
                # deadline: pool buffers (qb/yo/otn, 2-deep) must not be
                # re-granted before the previous generation's readers have
                # even been emitted.
                while budget > 0:
                    e = sched["open"]
                    if e is None:
                        if not queue or queue[0][0] > g + 16:
                            break
                        e = queue.pop(0)
                        sched["open"] = e
                    while budget > 0 and e[2] < len(e[1]):
                        cost, fn = e[1][e[2]]
                        e[2] += 1
                        fn()
                        budget -= cost
                    if e[2] == len(e[1]):
                        sched["open"] = None
                    else:
                        break

            # ---------- lead-in ----------
            for cost, fn in emit_kt_chain(0, 0):
                fn()
            for cost, fn in emit_q_chain(0, 0, qcells[0][0]):
                fn()
            for half in range(2):
                for cost, fn in emit_v_chain(0, half):
                    fn()

            # ---------- main fused loop ----------
            for it in range(IT):
                otn = sb2.tile([128, PAIRS, 512], BF16, tag="otn")
                for p in range(PAIRS):
                    b = it * 4 + p
                    qb = qcells[it][p]["qb"]
                    otA = ps_ot.tile([128, 512], F32, tag="otA")
                    otB = ps_ot.tile([128, 512], F32, tag="otB")

                    def scores(j, p=p, qb=qb):
                        stp = ps_sc.tile([128, 1024], F32, tag="sc", name="stp")
                        nc.tensor.matmul(
                            stp[:, 0:512], kT[0:64, p, bass.ts(j, 128)],
                            qb[0:64, :], start=True, stop=True,
                            tile_position=(0, 0),
                        )
                        nc.tensor.matmul(
                            stp[:, 512:1024], kT[64:128, p, bass.ts(j, 128)],
                            qb[64:128, :], start=True, stop=True,
                            tile_position=(64, 0),
                        )
                        ex = sb3.tile([128, 1024], BF16, tag="ex", name="ex")
                        with nc.allow_low_precision(reason="bf16 attn weights"):
                            nc.scalar.activation(ex[:], stp[:], EXP, scale=SCALE / (W8SCALE * W8SCALE))
                        return ex

                    ex_cur = scores(0)
                    # force due units (previous block's norm chain, late
                    # projections) BEFORE this block's first attnV: attnV j0
                    # overwrites otA/otB, which the norm must read first
                    sched_point(b * 16 - 1, 0)
                    for j in range(NT):
                        ex_nxt = scores(j + 1) if j + 1 < NT else None
                        nc.tensor.matmul(
                            otA[:, :], vaug[:, j, p, 0:128], ex_cur[:, 0:512],
                            start=(j == 0), stop=(j == NT - 1),
                        )
                        nc.tensor.matmul(
                            otB[:, :], vaug[:, j, p, 64:192], ex_cur[:, 512:1024],
                            start=(j == 0), stop=(j == NT - 1),
                        )
                        ex_cur = ex_nxt
                        sched_point(b * 16 + j, 560)

                    # softmax denominators -> srow (rows 64 / 0)
                    with nc.allow_low_precision(reason="f32r softmax sums"):
                        nc.vector.tensor_copy(srow[64:65, :], otA[64:65, :])
                        nc.vector.tensor_copy(srow[0:1, :], otB[0:1, :])
                    # norm reads otA/otB (single-buffered) -> force at the
                    # start of the next block, before its attnV j0 lands
                    add(b * 16 + 15, emit_norm_units(otA, otB, otn, p))

                # spread the output projection's 8 sub-chains across the
                # following blocks' slack instead of bursting at one
                # deadline. +16 keeps the earliest budget pop (deadline -
                # lookahead) after the forced norm(it, p3) that writes
                # otn[:, 3, :] at the next block's pre-attnV point.
                for idx, (isub, do) in enumerate(
                    (i, d) for i in range(4) for d in range(2)
                ):
                    add((it + 1) * 64 + 16 + 5 * idx,
                        emit_yproj_chain(it, otn, isub, do))

            # ---------- flush ----------
            if sched["open"] is not None:
                _drain(sched["open"])
                sched["open"] = None
            while queue:
                _drain(queue.pop(0))

    nc.compile()
    return nc


def _host_consts():
    # vaug cols 64:128 = [ones | zeros*63], shared by both heads' lhsT views
    vtpl = np.zeros((128, NT, PAIRS, 64), dtype=ml_dtypes.bfloat16)
    vtpl[:, :, :, 0] = 1.0
    sel = np.zeros((65, 128), dtype=np.float32)
    sel[64, 0:64] = 1.0     # rows 0-63  <- sums(even head)  (srow row 64)
    sel[0, 64:128] = 1.0    # rows 64-127 <- sums(odd head)  (srow row 0)
    zer = np.zeros((65, 512), dtype=np.float32)
    return vtpl, sel, zer


def kernel(x, Wq, Wk, Wv, Wo, bo, _run_kwargs=None):
    x = np.asarray(x, dtype=np.float32)
    Wq = np.asarray(Wq, dtype=np.float32)
    Wk = np.asarray(Wk, dtype=np.float32)
    Wv = np.asarray(Wv, dtype=np.float32)
    Wo = np.asarray(Wo, dtype=np.float32)
    bo = np.asarray(bo, dtype=np.float32)

    if "nc" not in _CACHE:
        _CACHE["nc"] = _build_kernel()
    nc = _CACHE["nc"]

    vtpl, sel, zer = _host_consts()
    in_maps = []
    for c in range(8):
        b, g = c // 2, c % 2
        xt = np.ascontiguousarray(x[b].T)
        cols = slice(g * DG, (g + 1) * DG)
        in_maps.append({
            "xbf": xt.astype(ml_dtypes.bfloat16),
            "xf8": xt.astype(ml_dtypes.float8_e4m3),
            "wq": np.ascontiguousarray(Wq[:, cols] * W8SCALE).astype(ml_dtypes.float8_e4m3),
            "wk": np.ascontiguousarray(Wk[:, cols] * W8SCALE).astype(ml_dtypes.float8_e4m3),
            "wv": np.ascontiguousarray(Wv[:, cols]).astype(ml_dtypes.bfloat16),
            "wo": np.ascontiguousarray(Wo[cols, :]).astype(ml_dtypes.bfloat16),
            "vtpl": vtpl,
            "sel": sel,
            "zer": zer,
        })

    res = bass_utils.run_bass_kernel_spmd(
        nc, in_maps, core_ids=list(range(8)), **(_run_kwargs or {})
    )
    if _run_kwargs:
        _CACHE["last_results"] = res

    y = np.empty((B, N, D), dtype=np.float32)
    for b in range(B):
        y[b] = res.results[2 * b]["y"] + res.results[2 * b + 1]["y"] + bo
    return y
